# revision 1
# baseline (speedup 1.0000x reference)
"""AttentionRoutingDetector on 8 NeuronCores, pure data parallel over batch.

Shards B=32 images/patches as 4 per core, replicates all weights, runs the
full forward (backbone convs -> attention routing -> dense experts + mask ->
aggregation -> detection heads) as one jitted program per core via pmap.
Returns full (cls_logits, reg_preds).
"""
import numpy as np
import jax
import jax.numpy as jnp
from jax import lax

TEMP = 1.0
N_CORES = 8

_ORDER = ['cw1', 'cb1', 'cw2', 'cb2', 'aw1', 'ab1', 'aw2', 'ab2', 'thr',
          'bw1', 'bb1', 'bw2', 'bb2', 'bw3', 'bb3', 'bw4', 'bb4',
          'sw1', 'sb1', 'sw2', 'sb2', 'gw', 'gb', 'dw1', 'db1',
          'cls_w', 'cls_b', 'reg_w', 'reg_b']


def _conv2d(x, w, b, stride):
    y = lax.conv_general_dilated(x, w, window_strides=(stride, stride),
                                 padding='SAME',
                                 dimension_numbers=('NCHW', 'OIHW', 'NCHW'))
    return y + b[None, :, None, None]


def _forward(images, patches, cw1, cb1, cw2, cb2, aw1, ab1, aw2, ab2, thr,
             bw1, bb1, bw2, bb2, bw3, bb3, bw4, bb4, sw1, sb1, sw2, sb2,
             gw, gb, dw1, db1, cls_w, cls_b, reg_w, reg_b):
    B = images.shape[0]
    x = jax.nn.relu(_conv2d(images, cw1, cb1, 2))
    x = jax.nn.relu(_conv2d(x, cw2, cb2, 2))
    f = x.reshape(B, 64, 16, 8, 16, 8).mean(axis=(3, 5))
    tok = f.reshape(B, 64, 256).transpose(0, 2, 1)
    h = jax.nn.relu(tok @ aw1 + ab1)
    attention_scores = jax.nn.sigmoid((h @ aw2 + ab2)[..., 0])
    soft = jax.nn.sigmoid((attention_scores - thr) / TEMP)
    mask = (soft > 0.5).astype(attention_scores.dtype)
    pf = patches.reshape(B, 256, -1)
    hb = jax.nn.relu(pf @ bw1 + bb1)
    hb = jax.nn.relu(hb @ bw2 + bb2)
    hb = jax.nn.relu(hb @ bw3 + bb3)
    high = (hb @ bw4 + bb4) * mask[..., None]
    hs = jax.nn.relu(pf @ sw1 + sb1)
    low = (hs @ sw2 + sb2) * (1.0 - mask)[..., None]
    comb = high + low
    mean_pool = comb.mean(axis=1)
    w = attention_scores / (attention_scores.sum(-1, keepdims=True) + 1e-6)
    attn_pool = jnp.einsum('bn,bnd->bd', w, comb)
    g = jax.nn.relu(jnp.concatenate([mean_pool, attn_pool], -1) @ gw + gb)
    fuse = jnp.concatenate(
        [jnp.broadcast_to(g[:, None, :], (B, 256, 256)), comb], -1)
    hd = jax.nn.relu(fuse @ dw1 + db1)
    cls_logits = hd @ cls_w + cls_b
    reg_preds = hd @ reg_w + reg_b
    return cls_logits, reg_preds


_PMAPPED = None


def _get_pmapped():
    global _PMAPPED
    if _PMAPPED is None:
        devs = jax.devices()[:N_CORES]
        _PMAPPED = jax.pmap(_forward, devices=devs,
                            in_axes=(0, 0) + (None,) * len(_ORDER))
    return _PMAPPED


def kernel(**inputs):
    images = np.asarray(inputs['images'], np.float32)
    patches = np.asarray(inputs['patches'], np.float32)
    B = images.shape[0]
    per = B // N_CORES
    imgs = images.reshape(N_CORES, per, *images.shape[1:])
    pats = patches.reshape(N_CORES, per, *patches.shape[1:])
    weights = [np.asarray(inputs[k], np.float32) for k in _ORDER]
    pm = _get_pmapped()
    cls, reg = pm(imgs, pats, *weights)
    cls = np.asarray(cls).reshape(B, 256, 80).astype(np.float32)
    reg = np.asarray(reg).reshape(B, 256, 4).astype(np.float32)
    return cls, reg
